# revision 61
# baseline (speedup 1.0000x reference)
"""Multi-head self-attention with positional bias, sharded over 8 NeuronCores.

Sharding: head-parallel. Core h computes head h for all batches; the full
output is the sum of the 8 per-core partials (row-parallel Wout), reduced on
host.

Device kernel (per core), fp16 matmul inputs / fp32 PSUM accumulation:
  - projections: packed q|k weight [d, 128] gives [128, 512]-psum chains
    (q rows 0-63, k rows 64-127); v accumulates 8 token tiles side by side
    per psum chunk. Batch 0 is projected up front; batches 1-3 are woven
    into the score stream, with evacuations on the Act engine (which is
    otherwise idle outside the exp stream).
  - scores are computed TRANSPOSED: ST[j, i] = k_j . q_i so exp's output is
    directly the layout the attention*V matmul needs.
  - the positional bias never touches the PE: host ships E = exp(bias^T) and
    the device computes P~ = exp(ST) * E with a 2x-mode fp16 DVE multiply.
  - softmax denominator: ones column appended to v; PV matmul row 64 then
    holds sum_j P~[j, i]. The reciprocal row is read straight from PSUM,
    partition-broadcast on GPSIMD, and the evacuate+normalize is one fused
    DVE scalar_tensor_tensor pass feeding the Wout matmuls.
  - emission is software-pipelined: engines dispatch in-order with a
    single-slot wait queue, so PV matmuls trail their exp/multiply chain by
    DEPTH steps, per-lb-sweep epilogues spread into following steps, and
    block order is pair-major (the first ~70us only needs batches 0/1).
  - PSUM: 3 double-bank score tiles + 2 single-bank PV accumulator slots
    (lb-major sweeps make the two accumulator pairs disjoint in time).
"""

import numpy as np
from contextlib import ExitStack

import concourse.bass as bass
import concourse.bacc as bacc
import concourse.mybir as mybir
import concourse.tile as tile
from concourse.bass_utils import run_bass_kernel_spmd

HEADS = 8
DH = 64
B, N, D = 4, 2048, 512
SCALE = DH ** -0.5
N_CORES = 8

F32 = mybir.dt.float32
F16 = mybir.dt.float16
MUL = mybir.AluOpType.mult


def build_nc(b=B, n=N, d=D, n_cores=1):
    """Per-core Bass program (SPMD: per-head differences come in via inputs)."""
    assert b % 2 == 0 and n % 512 == 0 and d % 128 == 0
    T = b * n
    CC = d // 128        # contraction chunks for projections
    NJ = n // 128        # key tiles (j)
    IC = 512
    NIC = n // IC        # i-chunks of 512
    NIP = NIC // 2       # i-groups of 1024
    NPAIR = b // 2
    VW = 2 * DH          # v block: DH values + DH ones columns (the PV
                         # matmul then emits the softmax denominator
                         # replicated on psum rows 64-127: no broadcast)

    nc = bacc.Bacc("TRN2", target_bir_lowering=False, debug=False,
                   num_devices=n_cores)
    qT = nc.declare_dram_parameter("qT", [d, T], F16, isOutput=False)
    eb = nc.declare_dram_parameter("eb", [n, n], F16, isOutput=False)
    wqk = nc.declare_dram_parameter("wqk", [d, 2 * DH], F16, isOutput=False)
    wv = nc.declare_dram_parameter("wv", [d, DH], F16, isOutput=False)
    wout = nc.declare_dram_parameter("wout", [DH, d], F16, isOutput=False)
    out = nc.declare_dram_parameter("out", [T, d], F16, isOutput=True)

    with ExitStack() as ctx:
        tc = ctx.enter_context(tile.TileContext(nc))

        const = ctx.enter_context(tc.tile_pool(name="const", bufs=1))
        qk_pool = ctx.enter_context(tc.tile_pool(name="qkT", bufs=1))
        v_pool = ctx.enter_context(tc.tile_pool(name="v", bufs=1))
        e_pool = ctx.enter_context(tc.tile_pool(name="ebias", bufs=1))
        ot_pool = ctx.enter_context(tc.tile_pool(name="otf", bufs=3))
        qt_pool = ctx.enter_context(tc.tile_pool(name="qt", bufs=8))
        p_pool = ctx.enter_context(tc.tile_pool(name="pexp", bufs=4))
        pr_pool = ctx.enter_context(tc.tile_pool(name="prod", bufs=8))
        osb_pool = ctx.enter_context(tc.tile_pool(name="osb", bufs=4))
        # PSUM: st_pool holds score tiles, projection accumulators and output
        # po tiles (all [128, 1024] f32 = 2 banks); ots holds PV accumulators.
        st_pool = ctx.enter_context(tc.tile_pool(name="st", bufs=3, space="PSUM"))
        ots_pool = ctx.enter_context(tc.tile_pool(name="ots", bufs=2, space="PSUM"))

        zbias = const.tile([128, 1], F32, tag="zbias")
        nc.vector.memset(zbias, 0.0)
        ones16 = const.tile([128, 16], F16, tag="ones16")
        nc.vector.memset(ones16, 1.0)

        wqk_sb = const.tile([128, CC, 2 * DH], F16, tag="wqk")
        nc.sync.dma_start(out=wqk_sb, in_=wqk[:, :].rearrange("(c p) e -> p c e", p=128))
        wv_sb = const.tile([128, CC, DH], F16, tag="wv")
        nc.sync.dma_start(out=wv_sb, in_=wv[:, :].rearrange("(c p) e -> p c e", p=128))
        wout_sb = const.tile([DH, d], F16, tag="wout")
        nc.sync.dma_start(out=wout_sb, in_=wout[:, :])

        qT_sb = [qk_pool.tile([DH, n], F16, tag=f"qT{bb}", name=f"qT{bb}") for bb in range(b)]
        kT_sb = [qk_pool.tile([DH, n], F16, tag=f"kT{bb}", name=f"kT{bb}") for bb in range(b)]
        v_sb = [v_pool.tile([128, NJ * VW], F16, tag=f"v{bb}", name=f"v{bb}") for bb in range(b)]
        for bb in range(b):
            ones_cols = v_sb[bb].rearrange("p (t w) -> p t w", w=VW)[:, :, DH:VW]
            nc.vector.memset(ones_cols, 1.0)



        # ---------------- projections (per batch) ----------------
        # DMA order matters (SP queue + DMA engines are serial): qt for the
        # first two batches goes out first so projections start immediately;
        # the E = exp(bias^T) staging streams behind it, and batches 2/3 are
        # loaded + projected while the first score block runs.
        e_sb = []

        def load_e_tiles(j0, j1):
            for jt in range(j0, j1):
                t = e_pool.tile([128, n], F16, tag=f"eb{jt}", name=f"eb{jt}")
                nc.sync.dma_start(out=t, in_=eb[jt * 128:(jt + 1) * 128, :])
                e_sb.append(t)

        def load_qt(bb, split=0):
            # split > 1: load in `split` column pieces per contraction chunk,
            # interleaved c-major, so the first projection unit can start
            # after 1/split of the batch's data has landed.
            qt_c = [qt_pool.tile([128, n], F16, tag="qt", name=f"qt{bb}_{c}")
                    for c in range(CC)]
            np_ = max(1, split)
            w = n // np_
            for p in range(np_):
                for c in range(CC):
                    nc.sync.dma_start(
                        out=qt_c[c][:, p * w:(p + 1) * w],
                        in_=qT[c * 128:(c + 1) * 128,
                               bb * n + p * w:bb * n + (p + 1) * w])
            return qt_c

        def emit_proj_qk(bb, qt_c, qh, act_evac=False):
            # q|k packed: psum rows 0-63 = q^T, 64-127 = k^T (one 512-wide
            # token chunk per unit so woven units disturb the score pipeline
            # as little as possible). Returns the evacuation closure.
            ps = st_pool.tile([128, 2 * IC], F32, tag="st", name=f"pqk{bb}_{qh}")
            acols = slice(qh * IC, (qh + 1) * IC)
            for c in range(CC):
                nc.tensor.matmul(ps[:, 0:IC], lhsT=wqk_sb[:, c, :],
                                 rhs=qt_c[c][:, acols],
                                 start=(c == 0), stop=(c == CC - 1),
                                 skip_group_check=True)

            def evac():
                if act_evac:  # the Act engine is idle in the lead-in
                    nc.scalar.copy(qT_sb[bb][:, acols], ps[0:DH, 0:IC])
                    nc.scalar.copy(kT_sb[bb][:, acols], ps[DH:128, 0:IC])
                else:
                    nc.vector.tensor_copy(qT_sb[bb][:, acols], ps[0:DH, 0:IC])
                    nc.vector.tensor_copy(kT_sb[bb][:, acols], ps[DH:128, 0:IC])
            return evac

        def emit_proj_v(bb, qt_c, vh, act_evac=False):
            # v: 8 token tiles side by side in one [128, 512] psum chunk
            HT = NJ // 2
            psv = st_pool.tile([128, 2 * IC], F32, tag="st", name=f"pv{bb}_{vh}")
            for t8 in range(HT):
                tt = vh * HT + t8
                for c in range(CC):
                    nc.tensor.matmul(psv[:, t8 * DH:(t8 + 1) * DH],
                                     lhsT=qt_c[c][:, tt * 128:(tt + 1) * 128],
                                     rhs=wv_sb[:, c, :],
                                     start=(c == 0), stop=(c == CC - 1),
                                     skip_group_check=True)
            vdst = v_sb[bb].rearrange("p (t w) -> p t w", w=VW)[:, vh * HT:(vh + 1) * HT, 0:DH]
            src = psv[:, 0:HT * DH].rearrange("p (t e) -> p t e", e=DH)

            def evac():
                if act_evac:
                    nc.scalar.copy(vdst, src)
                else:
                    nc.vector.tensor_copy(vdst, src)
            return evac

        qt_c = load_qt(0, split=4)
        load_e_tiles(0, 1)
        for qh in range(n // IC):
            emit_proj_qk(0, qt_c, qh, act_evac=True)()
        emit_proj_v(0, qt_c, 0, act_evac=True)()
        emit_proj_v(0, qt_c, 1, act_evac=True)()
        qt_w = {1: load_qt(1, split=2)}
        load_e_tiles(1, NJ)
        if b > 2:
            qt_w[2] = load_qt(2, split=2)
            qt_w[3] = load_qt(3, split=2)

        # ---------------- scores + softmax + P~^T V + out-proj ----------------
        # Software-pipelined emission: engines dispatch in-order with a
        # single-slot wait queue, so PV matmuls are emitted DEPTH steps after
        # their qk/exp/prod chain, and the block epilogue (evac, reciprocal,
        # normalize, Wout matmuls, store) is spread into the next block's
        # steps. This keeps the PE/Act queues free of head-of-line stalls.
        exp_fn = mybir.ActivationFunctionType.Exp
        PROD_POOL_JTS = frozenset()  # pool multiply is too slow for the PV path
        DEPTH, POOL_DEPTH = 4, 6

        steps = [(ip, pair, jt, lb)
                 for pair in range(NPAIR) for ip in range(NIP)
                 for lb in range(2) for jt in range(NJ)]
        SPB = NJ * 2  # steps per (ip, pair) block

        ot_ps_blk = {}     # block index -> {(lb, il): psum tile}
        pv_q = []          # (release_step, fn)
        extra_q = []       # (release_step, fn)

        def emit_pv(blk, pair, jt, lb, prod):
            def fn():
                bb = 2 * pair + lb
                for il in range(2):
                    nc.tensor.matmul(
                        ot_ps_blk[blk][(lb, il)],
                        lhsT=v_sb[bb][:, jt * VW:jt * VW + VW],
                        rhs=prod[:, il * IC:(il + 1) * IC],
                        start=(jt == 0), stop=(jt == NJ - 1),
                        skip_group_check=True)
            return fn

        def emit_evac(blk, ip, pair, lb, il):
            # reciprocal of the (replicated) denominator rows, straight
            # from PSUM -- already broadcast across partitions
            def fn():
                rr = ot_pool.tile([DH, IC], F16, tag="rr", name="rr")
                with nc.allow_low_precision("fp16 softmax denom reciprocal"):
                    nc.vector.reciprocal(rr, ot_ps_blk[blk][(lb, il)][DH:VW, :])
                of_blk[(blk, lb, il)] = rr
            return fn

        def emit_norm(blk, lb, il):
            # fused evacuate+normalize: onrm = psum * recip (one DVE pass)
            def fn():
                rr = of_blk[(blk, lb, il)]
                onrm = ot_pool.tile([DH, IC], F16, tag="on", name="on")
                nc.vector.scalar_tensor_tensor(
                    onrm, ot_ps_blk[blk][(lb, il)][0:DH, :], 1.0, rr, MUL, MUL)
                onorm_blk[(blk, lb, il)] = onrm
            return fn

        def emit_po(blk, ip, pair, lb, il, tp, last):
            def fn():
                bb = 2 * pair + lb
                ic = ip * 2 + il
                onrm = onorm_blk[(blk, lb, il)]
                po = st_pool.tile([128, 2 * IC], F32, tag="st", name="po")
                for q in range(2):
                    off = (tp * 2 + q) * 128
                    nc.tensor.matmul(
                        po[:, q * d:(q + 1) * d],
                        lhsT=onrm[:, off:off + 128], rhs=wout_sb,
                        start=True, stop=True, skip_group_check=True)
                osb = osb_pool.tile([128, 2 * IC], F16, tag="osb")
                if last:  # tail: the Act engine is idle by then
                    nc.scalar.copy(osb[:, 0:2 * d], po[:, 0:2 * d])
                else:
                    nc.vector.tensor_copy(osb[:, 0:2 * d], po[:, 0:2 * d])
                r0 = bb * n + ic * IC + tp * 256
                nc.sync.dma_start(
                    out=out[r0:r0 + 256, :].rearrange("(t p) d -> p t d", p=128),
                    in_=osb[:, 0:2 * d].rearrange("p (t d) -> p t d", t=2))
            return fn

        onorm_blk = {}
        of_blk = {}
        n_steps = len(steps)
        n_blk = n_steps // SPB
        last_pv_rel = {}   # (blk, lb) -> last release step (keeps psum order)
        # proj weave: (batch, unit) with units 0..NQH-1 = qk chunks, then 2 v halves
        NQH = n // IC
        proj_w = {3 + 2 * u: (1, u) for u in range(NQH + 2)}
        if b > 2:
            proj_w.update({44 + 4 * u: (2, u) for u in range(NQH + 2)})
            proj_w.update({58 + 4 * u: (3, u) for u in range(NQH + 2)})
        for s in range(n_steps + POOL_DEPTH + 20):
            if s in proj_w:  # weave remaining projections into early score steps
                pb, part = proj_w[s]
                # matmuls go out now; the psum evacuation is released at the
                # end of the NEXT step so it queues behind the critical
                # exp->prod chain of the current steps
                if part < NQH:
                    ev = emit_proj_qk(pb, qt_w[pb], part, act_evac=True)
                else:
                    ev = emit_proj_v(pb, qt_w[pb], part - NQH, act_evac=True)
                extra_q.append((s + 1, ev))
            if s < n_steps:
                ip, pair, jt, lb = steps[s]
                blk = s // SPB
                bstart = blk * SPB
                if jt == 0:
                    if s % SPB == 0:
                        ot_ps_blk[blk] = {}
                    for i2 in range(2):
                        ot_ps_blk[blk][(lb, i2)] = ots_pool.tile(
                            [128, IC], F32, tag="ot", name="otp")
                bb = 2 * pair + lb
                st = st_pool.tile([128, 2 * IC], F32, tag="st", name="st")
                for il in range(2):
                    ic = ip * 2 + il
                    nc.tensor.matmul(
                        st[:, il * IC:(il + 1) * IC],
                        lhsT=kT_sb[bb][:, jt * 128:(jt + 1) * 128],
                        rhs=qT_sb[bb][:, ic * IC:(ic + 1) * IC],
                        start=True, stop=True, skip_group_check=True)
                pexp = p_pool.tile([128, 2 * IC], F16, tag="pexp")
                nc.scalar.activation(pexp, st, exp_fn, bias=zbias)
                prod = pr_pool.tile([128, 2 * IC], F16, tag="prod")
                on_pool = blk >= 2 and jt in PROD_POOL_JTS
                peng = nc.gpsimd if on_pool else nc.vector
                peng.tensor_tensor(
                    prod, pexp, e_sb[jt][:, ip * 2 * IC:(ip + 1) * 2 * IC], MUL)
                rel = s + (POOL_DEPTH if on_pool else DEPTH)
                if jt == 0:
                    # the slot this PV resets is freed by the previous
                    # sweep's normalize; don't park it at the queue head
                    rel += 3
                # psum group order: start-matmul first, stop-matmul last
                rel = max(rel, last_pv_rel.get((blk, lb), 0))
                last_pv_rel[(blk, lb)] = rel
                pv_q.append((rel, emit_pv(blk, pair, jt, lb, prod)))
                if jt == NJ - 1:  # this lb sweep done: schedule its epilogue
                    base = s + DEPTH  # right at the sweep's last PV
                    tail = blk == n_blk - 1 and lb == 1
                    for i2 in range(2):
                        if tail:  # compress: nothing left to overlap with
                            extra_q.append((base, emit_evac(blk, ip, pair, lb, i2)))
                            extra_q.append((base + 1, emit_norm(blk, lb, i2)))
                            for tp in range(2):
                                extra_q.append((base + 2,
                                                emit_po(blk, ip, pair, lb, i2, tp, True)))
                        else:
                            extra_q.append((base + i2, emit_evac(blk, ip, pair, lb, i2)))
                            extra_q.append((base + 1 + i2, emit_norm(blk, lb, i2)))
                            for tp in range(2):
                                extra_q.append((base + 3 + 2 * i2 + tp,
                                                emit_po(blk, ip, pair, lb, i2, tp, False)))
            for q in (pv_q, extra_q):
                ready = [f for r, f in q if r <= s]
                q[:] = [(r, f) for r, f in q if r > s]
                for f in ready:
                    f()
    nc.compile()
    return nc


def make_in_maps(query, pos_bias, Wq, Wk, Wv, Wout, n_cores=N_CORES):
    """Host-side sharding/layout prep. Head h -> core h."""
    query = np.asarray(query, dtype=np.float32)
    pos_bias = np.asarray(pos_bias, dtype=np.float32)
    Wq = np.asarray(Wq, dtype=np.float32)
    Wk = np.asarray(Wk, dtype=np.float32)
    Wv = np.asarray(Wv, dtype=np.float32)
    Wout = np.asarray(Wout, dtype=np.float32)

    b, n, d = query.shape
    qT = np.ascontiguousarray(query.reshape(b * n, d).T.astype(np.float16))
    wq_s = Wq * np.float32(SCALE)
    in_maps = []
    for h in range(n_cores):
        sl = slice(h * DH, (h + 1) * DH)
        in_maps.append({
            "qT": qT,
            "eb": np.ascontiguousarray(np.exp(pos_bias[h].T).astype(np.float16)),
            "wqk": np.ascontiguousarray(
                np.concatenate([wq_s[:, sl], Wk[:, sl]], axis=1).astype(np.float16)),
            "wv": np.ascontiguousarray(Wv[:, sl].astype(np.float16)),
            "wout": np.ascontiguousarray(Wout[sl, :].astype(np.float16)),
        })
    return in_maps


def run_device(in_maps, b=B, n=N, d=D, trace=False, **kw):
    nc = build_nc(b, n, d, n_cores=len(in_maps))
    return run_bass_kernel_spmd(nc, in_maps, list(range(len(in_maps))), trace=trace, **kw)


def assemble(results, b=B, n=N, d=D):
    acc = np.zeros((b * n, d), dtype=np.float32)
    for r in results:
        acc += r["out"]
    return acc.reshape(b, n, d)


def kernel(query, pos_bias, Wq, Wk, Wv, Wout):
    in_maps = make_in_maps(query, pos_bias, Wq, Wk, Wv, Wout)
    res = run_device(in_maps)
    return assemble(res.results)
